# revision 7
# baseline (speedup 1.0000x reference)
"""MultiHeadCoAttention Trainium2 Bass kernel, 8-way head-parallel SPMD.

Contract: kernel(**inputs) takes the full (unsharded) inputs of the reference
nn.Module and returns the full output tuple (out_q, out_c).

Sharding strategy (hardcoded for B=2, Lq=Lc=2048, D=1024, H=16, dk=64, 8 cores):
  - core k owns heads {2k, 2k+1} for both batches (head-parallel attention);
  - all four input projections, scores, the two softmaxes and both attention
    applies for those heads run fully on-core with no communication;
  - softmax is computed max-free (scores are O(5), exp is exact in fp32) with
    the row/col sums obtained for free as an extra ones-column in the value
    matmuls, so only one exp pass per score orientation is needed;
  - the col-softmax orientation exp(S) is either recomputed (scores+exp) or,
    for half the pairs, produced by transposing the row-orientation exp(S^T)
    through a DRAM round-trip on the (otherwise idle) DMA engines — trading
    ScalarE exp time for DMA bandwidth;
  - two on-device AllToAlls redistribute per-head results from
    [d-slice, all tokens] to [all d, token-slice]; the q-side one fires as
    soon as the row-softmax half is done, so its latency and the whole out_q
    output projection hide under the col-softmax compute;
  - host side only slices/casts weights and concatenates the 8 token-slices.
Compute dtype is fp16 (PE runs fp16 at full rate vs 4x slower fp32) with fp32
PSUM accumulation everywhere.
"""

import numpy as np

B, LQ, LC, D, H, DK = 2, 2048, 2048, 1024, 16, 64
N_CORES = 8
HPC = H // N_CORES          # heads per core = 2
DSL = HPC * DK              # d-slice width per core = 128
LTOT = B * LQ               # 4096 flattened token rows
LSL = LTOT // N_CORES       # 512 token rows per core
NKT = D // 128              # 8 k-tiles over the model dim
NLT = LQ // 128             # 16 l-tiles per batch
VW = DK + 1                 # value tile width incl ones column
SCALE = 1.0 / float(np.sqrt(DK))
# (b, h) pairs whose col-softmax matrix is produced by DMA-transposing the
# row-softmax exp instead of a second scores+exp pass
OFFLOAD = {(0, 0), (1, 0)}

_CACHE = {}


def _build_program():
    import concourse.bacc as bacc
    import concourse.mybir as mybir
    from concourse import tile

    f32 = mybir.dt.float32
    f16 = mybir.dt.float16
    Exp = mybir.ActivationFunctionType.Exp
    add = mybir.AluOpType.add
    mult = mybir.AluOpType.mult

    nc = bacc.Bacc("TRN2", target_bir_lowering=False, debug=False,
                   num_devices=N_CORES)

    # ---- I/O ----
    query = nc.dram_tensor("query", [B, LQ, D], f32, kind="ExternalInput")
    context = nc.dram_tensor("context", [B, LC, D], f32, kind="ExternalInput")
    w0t = nc.dram_tensor("w0t", [D, DSL], f16, kind="ExternalInput")
    w1t = nc.dram_tensor("w1t", [D, DSL], f16, kind="ExternalInput")
    w2t = nc.dram_tensor("w2t", [D, DSL], f16, kind="ExternalInput")
    w3t = nc.dram_tensor("w3t", [D, DSL], f16, kind="ExternalInput")
    w4t = nc.dram_tensor("w4t", [D, D], f16, kind="ExternalInput")
    w5t = nc.dram_tensor("w5t", [D, D], f16, kind="ExternalInput")
    b0s = nc.dram_tensor("b0s", [DSL, 1], f32, kind="ExternalInput")
    b1s = nc.dram_tensor("b1s", [DSL, 1], f32, kind="ExternalInput")
    b2r = nc.dram_tensor("b2r", [128, DSL], f32, kind="ExternalInput")
    b3r = nc.dram_tensor("b3r", [128, DSL], f32, kind="ExternalInput")
    b4r = nc.dram_tensor("b4r", [128, D], f32, kind="ExternalInput")
    b5r = nc.dram_tensor("b5r", [128, D], f32, kind="ExternalInput")
    ident = nc.dram_tensor("ident", [128, 128], f16, kind="ExternalInput")
    out0c = nc.dram_tensor("out0c", [LSL, D], f32, kind="ExternalOutput")
    out1c = nc.dram_tensor("out1c", [LSL, D], f32, kind="ExternalOutput")

    with tile.TileContext(nc) as tc:
        with tc.tile_pool(name="dram", bufs=1, space="DRAM") as dram, \
             tc.tile_pool(name="const", bufs=1) as constp, \
             tc.tile_pool(name="psA", bufs=3, space="PSUM") as psA, \
             tc.tile_pool(name="psB", bufs=2, space="PSUM") as psB:

            # fp16 staging of the two activations (cast on SWDGE)
            stage_q = dram.tile([B, LQ, D], f16)
            stage_c = dram.tile([B, LC, D], f16)
            a2aq_in = dram.tile([N_CORES, DSL, LSL], f16)
            a2aq_out = dram.tile([N_CORES, DSL, LSL], f16)
            a2ac_in = dram.tile([N_CORES, DSL, LSL], f16)
            a2ac_out = dram.tile([N_CORES, DSL, LSL], f16)
            # DRAM bounce for the transpose-offloaded exp matrices
            etd = {bh: dram.tile([LC, LQ], f16, name=f"etd{bh[0]}_{bh[1]}")
                   for bh in OFFLOAD}

            for b in range(B):
                nc.gpsimd.dma_start(stage_q[b], query.ap()[b])
                nc.gpsimd.dma_start(stage_c[b], context.ap()[b])

            # constants / weights on the sync queue, emitted before the input
            # transposes (they fit in the window while the first cast runs)
            idt = constp.tile([128, 128], f16, name="idt")
            nc.sync.dma_start(idt[:], ident.ap())
            bias_qp = constp.tile([DSL, 1], f32, name="bias_qp")
            nc.sync.dma_start(bias_qp[:], b0s.ap())
            bias_cp = constp.tile([DSL, 1], f32, name="bias_cp")
            nc.sync.dma_start(bias_cp[:], b1s.ap())
            bias_qv = constp.tile([128, DSL], f32, name="bias_qv")
            nc.sync.dma_start(bias_qv[:], b2r.ap())
            bias_cv = constp.tile([128, DSL], f32, name="bias_cv")
            nc.sync.dma_start(bias_cv[:], b3r.ap())
            wq = [constp.tile([128, DSL], f16, name=f"wq{k}") for k in range(NKT)]
            wc = [constp.tile([128, DSL], f16, name=f"wc{k}") for k in range(NKT)]
            wqv = [constp.tile([128, DSL], f16, name=f"wqv{k}") for k in range(NKT)]
            wcv = [constp.tile([128, DSL], f16, name=f"wcv{k}") for k in range(NKT)]
            for k in range(NKT):
                sl = slice(128 * k, 128 * (k + 1))
                nc.sync.dma_start(wq[k][:], w0t.ap()[sl])
                nc.sync.dma_start(wc[k][:], w1t.ap()[sl])
                nc.sync.dma_start(wqv[k][:], w2t.ap()[sl])
                nc.sync.dma_start(wcv[k][:], w3t.ap()[sl])

            # ---- phase 1: transpose inputs + all projections ----
            with tc.tile_pool(name="proj", bufs=1) as projp:
                qTp = [projp.tile([128, LQ], f16, name=f"qTp{b}") for b in range(B)]
                cTp = [projp.tile([128, LC], f16, name=f"cTp{b}") for b in range(B)]
                # merged per-(batch, ltile) value tiles: cols [0:65] head 0
                # (ones at 64), [65:130] head 1 (ones at 129)
                qvv = [[projp.tile([128, 2 * VW], f16, name=f"qvv{b}_{lt}")
                        for lt in range(NLT)] for b in range(B)]
                cvv = [[projp.tile([128, 2 * VW], f16, name=f"cvv{b}_{lt}")
                        for lt in range(NLT)] for b in range(B)]

                with tc.tile_pool(name="inT", bufs=2 * NKT) as inp:
                    for b in range(B):
                        qT = [inp.tile([128, LQ], f16, tag="qT", name=f"qT{b}_{k}")
                              for k in range(NKT)]
                        cT = [inp.tile([128, LC], f16, tag="cT", name=f"cT{b}_{k}")
                              for k in range(NKT)]
                        for k in range(NKT):
                            dsl = slice(128 * k, 128 * (k + 1))
                            nc.sync.dma_start(qT[k][:], stage_q[b, :, dsl],
                                              transpose=True)
                            nc.sync.dma_start(cT[k][:], stage_c[b, :, dsl],
                                              transpose=True)
                        for (dst, w_, src, bias) in ((cTp, wc, cT, bias_cp),
                                                     (qTp, wq, qT, bias_qp)):
                            for ch in range(LQ // 512):
                                cs = slice(512 * ch, 512 * (ch + 1))
                                ps = psB.tile([128, 512], f32, tag="pss", name="ps")
                                for k in range(NKT):
                                    nc.tensor.matmul(ps[:], w_[k][:], src[k][:, cs],
                                                     start=(k == 0),
                                                     stop=(k == NKT - 1))
                                nc.vector.tensor_scalar(
                                    out=dst[b][:, cs], in0=ps[:],
                                    scalar1=bias[:, 0:1], scalar2=None, op0=add)
                        for (dst, w_, src, bias) in ((cvv, wcv, cT, bias_cv),
                                                     (qvv, wqv, qT, bias_qv)):
                            for lt in range(NLT):
                                ls = slice(128 * lt, 128 * (lt + 1))
                                ps = psB.tile([128, DSL], f32, tag="pss", name="ps")
                                for k in range(NKT):
                                    nc.tensor.matmul(ps[:], src[k][:, ls], w_[k][:],
                                                     start=(k == 0),
                                                     stop=(k == NKT - 1))
                                t = dst[b][lt]
                                for h in range(HPC):
                                    hs = slice(DK * h, DK * (h + 1))
                                    os = slice(VW * h, VW * h + DK)
                                    nc.vector.tensor_tensor(
                                        out=t[:, os], in0=ps[:, hs],
                                        in1=bias[:, hs], op=add)
                                    nc.vector.memset(
                                        t[:, VW * h + DK:VW * (h + 1)], 1.0)

                # ---- phase 2: attention ----
                with tc.tile_pool(name="att", bufs=1) as attp, \
                     tc.tile_pool(name="emat", bufs=48) as ematp:
                    rq = [[attp.tile([128, 128], f16, name=f"rq{b}_{m}")
                           for m in range(NLT)] for b in range(B)]
                    rc = [[attp.tile([128, 128], f16, name=f"rc{b}_{m}")
                           for m in range(NLT)] for b in range(B)]
                    rqt = [attp.tile([128, LQ], f16, name=f"rqt{b}") for b in range(B)]
                    rct = [attp.tile([128, LC], f16, name=f"rct{b}") for b in range(B)]

                    def scores_exp_packed(b, lhsp, rhsp, qh, dumps):
                        """Both heads' exp(S/sqrt(dk)) for one q-half, the two
                        K=64 score matmuls packed into PE row groups 0/64."""
                        ets = ([], [])
                        for kt in range(NLT):
                            ks = slice(128 * kt, 128 * (kt + 1))
                            sps = [psA.tile([128, 1024], f32, tag="sps", name="sp")
                                   for _ in range(HPC)]
                            for cch in range(2):
                                c0 = 1024 * qh + 512 * cch
                                ds = slice(512 * cch, 512 * (cch + 1))
                                for h in range(HPC):
                                    hp = slice(64 * h, 64 * (h + 1))
                                    nc.tensor.matmul(
                                        sps[h][:, ds], lhsp[hp, ks],
                                        rhsp[hp, c0:c0 + 512],
                                        start=True, stop=True)
                            for h in range(HPC):
                                e = ematp.tile([128, 1024], f16, tag="et", name="e")
                                nc.scalar.activation(e[:], sps[h][:], Exp,
                                                     scale=SCALE)
                                if dumps[h] is not None:
                                    nc.gpsimd.dma_start(
                                        dumps[h][ks, 1024 * qh:1024 * (qh + 1)],
                                        e[:])
                                ets[h].append(e)
                        return ets

                    def scores_exp_half(b, h, lhsp, rhsp, qh):
                        """One head's exp tiles for one half (V phase)."""
                        hp = slice(64 * h, 64 * (h + 1))
                        et = []
                        for kt in range(NLT):
                            ks = slice(128 * kt, 128 * (kt + 1))
                            sp = psA.tile([128, 1024], f32, tag="sps", name="sp")
                            for cch in range(2):
                                c0 = 1024 * qh + 512 * cch
                                ds = slice(512 * cch, 512 * (cch + 1))
                                nc.tensor.matmul(sp[:, ds], lhsp[hp, ks],
                                                 rhsp[hp, c0:c0 + 512],
                                                 start=True, stop=True)
                            e = ematp.tile([128, 1024], f16, tag="et", name="e")
                            nc.scalar.activation(e[:], sp[:], Exp, scale=SCALE)
                            et.append(e)
                        return et

                    def xbar_half(bh, ch):
                        """V-phase E tiles for one c-half via xbar transpose."""
                        et = []
                        for kt in range(NLT):
                            ks = slice(128 * kt, 128 * (kt + 1))
                            e = ematp.tile([128, 1024], f16, tag="et", name="e")
                            nc.sync.dma_start(
                                e[:], etd[bh][1024 * ch:1024 * (ch + 1), ks],
                                transpose=True)
                            et.append(e)
                        return et

                    def apply_norm_half(et, vals, h, rdst, mh):
                        hp = slice(64 * h, 64 * (h + 1))
                        vs = slice(VW * h, VW * (h + 1))
                        for mi in range(8):
                            m = 8 * mh + mi
                            up = psB.tile([128, VW], f32, tag="pss", name="up")
                            for kt in range(NLT):
                                nc.tensor.matmul(
                                    up[:], et[kt][:, 128 * mi:128 * (mi + 1)],
                                    vals[kt][:, vs],
                                    start=(kt == 0), stop=(kt == NLT - 1))
                            rec = attp.tile([128, 1], f32, tag="rec", bufs=4,
                                            name="rec")
                            nc.vector.reciprocal(rec[:], up[:, DK:DK + 1])
                            nc.vector.tensor_scalar(
                                out=rdst[m][:, hp], in0=up[:, 0:DK],
                                scalar1=rec[:, 0:1], scalar2=None, op0=mult)

                    def shard_out(r, rt, b, a2a_in):
                        for m in range(NLT):
                            ms = slice(128 * m, 128 * (m + 1))
                            tp = psB.tile([128, 128], f16, tag="pss", name="tp")
                            nc.tensor.transpose(tp[:], r[b][m][:], idt[:])
                            nc.vector.tensor_copy(rt[b][:, ms], tp[:])
                        for j in range(4):
                            js = slice(512 * j, 512 * (j + 1))
                            nc.gpsimd.dma_start(a2a_in[4 * b + j], rt[b][:, js])

                    # U phase (row softmax -> q_res), both heads packed in the
                    # PE array; exp(S^T) halves of offloaded pairs go to DRAM
                    for b in range(B):
                        dumps = [etd.get((b, h)) for h in range(HPC)]
                        for qh in range(2):
                            ets = scores_exp_packed(b, cTp[b], qTp[b], qh, dumps)
                            for h in range(HPC):
                                apply_norm_half(ets[h], cvv[b], h, rq[b], qh)
                        shard_out(rq, rqt, b, a2aq_in)

                    nc.gpsimd.collective_compute(
                        "AllToAll", mybir.AluOpType.bypass,
                        replica_groups=[list(range(N_CORES))],
                        ins=[a2aq_in.opt()], outs=[a2aq_out.opt()])

                    # V phase (col softmax -> c_res), non-offloaded head first
                    for b in range(B):
                        for h in sorted(range(HPC),
                                        key=lambda h_: (b, h_) in OFFLOAD):
                            for ch in range(2):
                                if (b, h) in OFFLOAD:
                                    et = xbar_half((b, h), ch)
                                else:
                                    et = scores_exp_half(b, h, qTp[b], cTp[b], ch)
                                apply_norm_half(et, qvv[b], h, rc[b], ch)
                        shard_out(rc, rct, b, a2ac_in)

                    # out_q projection, hidden under the V phase: weights are
                    # streamed from DRAM through a small rotating pool
                    with tc.tile_pool(name="o0p", bufs=1) as o0p:
                        rqf = [o0p.tile([128, LSL], f16, name=f"rqf{k}")
                               for k in range(NKT)]
                        for k in range(NKT):
                            nc.gpsimd.dma_start(rqf[k][:], a2aq_out[k])
                        bias4 = o0p.tile([128, D], f32, name="bias4")
                        nc.gpsimd.dma_start(bias4[:], b4r.ap())
                        for mt in range(LSL // 128):
                            ms = slice(128 * mt, 128 * (mt + 1))
                            for ch in range(D // 512):
                                cs = slice(512 * ch, 512 * (ch + 1))
                                ps = psB.tile([128, 512], f32, tag="pss", name="ps")
                                for k in range(NKT):
                                    wk = o0p.tile([128, 512], f16, tag="w4s",
                                                  bufs=4, name="wk")
                                    nc.sync.dma_start(
                                        wk[:], w4t.ap()[128 * k:128 * (k + 1), cs])
                                    nc.tensor.matmul(ps[:], rqf[k][:, ms], wk[:],
                                                     start=(k == 0),
                                                     stop=(k == NKT - 1))
                                ev = o0p.tile([128, 512], f32, tag="oev", bufs=3,
                                              name="ev")
                                nc.vector.tensor_tensor(out=ev[:], in0=ps[:],
                                                        in1=bias4[:, cs], op=add)
                                nc.gpsimd.dma_start(out0c.ap()[ms, cs], ev[:])

                    nc.gpsimd.collective_compute(
                        "AllToAll", mybir.AluOpType.bypass,
                        replica_groups=[list(range(N_CORES))],
                        ins=[a2ac_in.opt()], outs=[a2ac_out.opt()])

            # ---- phase 3: out_c projection ----
            with tc.tile_pool(name="outp", bufs=1) as outp:
                w5 = [outp.tile([128, D], f16, name=f"w5_{k}") for k in range(NKT)]
                bias5 = outp.tile([128, D], f32, name="bias5")
                nc.sync.dma_start(bias5[:], b5r.ap())
                for k in range(NKT):
                    sl = slice(128 * k, 128 * (k + 1))
                    nc.sync.dma_start(w5[k][:], w5t.ap()[sl])
                rcf = [outp.tile([128, LSL], f16, name=f"rcf{k}") for k in range(NKT)]
                for k in range(NKT):
                    nc.sync.dma_start(rcf[k][:], a2ac_out[k])

                for mt in range(LSL // 128):
                    ms = slice(128 * mt, 128 * (mt + 1))
                    for ch in range(D // 512):
                        cs = slice(512 * ch, 512 * (ch + 1))
                        ps = psB.tile([128, 512], f32, tag="pss", name="ps")
                        for k in range(NKT):
                            nc.tensor.matmul(ps[:], rcf[k][:, ms], w5[k][:, cs],
                                             start=(k == 0), stop=(k == NKT - 1))
                        ev = outp.tile([128, 512], f32, tag="oev", bufs=3,
                                       name="ev")
                        nc.vector.tensor_tensor(out=ev[:], in0=ps[:],
                                                in1=bias5[:, cs], op=add)
                        nc.sync.dma_start(out1c.ap()[ms, cs], ev[:])

    nc.compile()
    return nc


def _prep_inputs(inputs):
    f16 = np.float16
    f32 = np.float32
    q = np.ascontiguousarray(np.asarray(inputs["query"], dtype=f32))
    c = np.ascontiguousarray(np.asarray(inputs["context"], dtype=f32))
    W = [np.asarray(inputs[f"W{i}"], dtype=f32) for i in range(6)]
    bias = [np.asarray(inputs[f"b{i}"], dtype=f32) for i in range(6)]
    ident = np.eye(128, dtype=f16)
    in_maps = []
    for k in range(N_CORES):
        dsl = slice(DSL * k, DSL * (k + 1))
        m = {
            "query": q,
            "context": c,
            "w0t": np.ascontiguousarray(W[0][dsl].T.astype(f16)),
            "w1t": np.ascontiguousarray(W[1][dsl].T.astype(f16)),
            "w2t": np.ascontiguousarray(W[2][dsl].T.astype(f16)),
            "w3t": np.ascontiguousarray(W[3][dsl].T.astype(f16)),
            "w4t": np.ascontiguousarray(W[4].T.astype(f16)),
            "w5t": np.ascontiguousarray(W[5].T.astype(f16)),
            "b0s": np.ascontiguousarray(bias[0][dsl].reshape(DSL, 1)),
            "b1s": np.ascontiguousarray(bias[1][dsl].reshape(DSL, 1)),
            "b2r": np.ascontiguousarray(np.tile(bias[2][dsl], (128, 1))),
            "b3r": np.ascontiguousarray(np.tile(bias[3][dsl], (128, 1))),
            "b4r": np.ascontiguousarray(np.tile(bias[4], (128, 1))),
            "b5r": np.ascontiguousarray(np.tile(bias[5], (128, 1))),
            "ident": ident,
        }
        in_maps.append(m)
    return in_maps


def _get_program():
    if "nc" not in _CACHE:
        _CACHE["nc"] = _build_program()
    return _CACHE["nc"]


def kernel(**inputs):
    from concourse.bass_utils import run_bass_kernel_spmd

    nc = _get_program()
    in_maps = _prep_inputs(inputs)
    res = run_bass_kernel_spmd(nc, in_maps, list(range(N_CORES)))
    out0 = np.concatenate([res.results[k]["out0c"] for k in range(N_CORES)], axis=0)
    out1 = np.concatenate([res.results[k]["out1c"] for k in range(N_CORES)], axis=0)
    return (out0.reshape(B, LQ, D).astype(np.float32),
            out1.reshape(B, LC, D).astype(np.float32))


# revision 13
# speedup vs baseline: 14070.6562x; 14070.6562x over previous
"""MultiHeadCoAttention Trainium2 Bass kernel, 8-way head-parallel SPMD.

kernel(**inputs) takes the full (unsharded) inputs of the reference nn.Module
and returns the full output tuple (out_q, out_c).

Sharding (hardcoded for B=2, Lq=Lc=2048, D=1024, H=16, dk=64, 8 NeuronCores):
  - core k owns heads {2k, 2k+1} for both batches (head-parallel attention);
    projections, scores, both softmaxes and both attention applies for those
    heads run fully on-core with no communication;
  - softmax is computed max-free (scores are O(5) so exp is exact in fp32);
    the row/col sums come for free as an extra ones-column in the value
    matmuls, so only one exp pass per score orientation is needed;
  - the two K=64 score matmuls (one per head) are packed into PE row groups
    0/64 and run concurrently in the systolic array (measured 2.55x);
  - the col-softmax orientation exp(S) is recomputed for one head and, for
    the other, produced by transposing the row-orientation exp(S^T) through
    a DRAM round-trip on otherwise-idle DMA engines (trades ScalarE exp time
    for DMA bandwidth);
  - two on-device AllToAlls redistribute per-head results from
    [d-slice, all tokens] to [all d, token-slice]; the q-side one fires as
    soon as the row-softmax half is done so its latency and the entire out_q
    output projection hide under the col-softmax compute;
  - each core computes the output linears for its 512-token slice only; the
    host slices/casts weights and concatenates the 8 token-slices.
Compute dtype is fp16 (PE runs fp16 at full rate vs 4x slower fp32) with
fp32 PSUM accumulation everywhere; end-to-end error vs the fp32 reference is
~7e-4 relative.
"""

import numpy as np

B, LQ, LC, D, H, DK = 2, 2048, 2048, 1024, 16, 64
N_CORES = 8
HPC = H // N_CORES          # heads per core = 2
DSL = HPC * DK              # d-slice width per core = 128
LTOT = B * LQ               # 4096 flattened token rows
LSL = LTOT // N_CORES       # 512 token rows per core
NKT = D // 128              # 8 k-tiles over the model dim
NLT = LQ // 128             # 16 l-tiles per batch
VW = DK + 1                 # value tile width incl ones column
SCALE = 1.0 / float(np.sqrt(DK))
# (b, h) pairs whose col-softmax matrix comes from DMA-transposing the
# row-softmax exp instead of a second scores+exp pass
OFFLOAD = {(0, 0), (1, 0)}

_CACHE = {}


def _build_program(reps=1):
    import concourse.bacc as bacc
    import concourse.mybir as mybir
    from concourse import tile

    f32 = mybir.dt.float32
    f16 = mybir.dt.float16
    Exp = mybir.ActivationFunctionType.Exp
    add = mybir.AluOpType.add
    mult = mybir.AluOpType.mult

    nc = bacc.Bacc("TRN2", target_bir_lowering=False, debug=False,
                   num_devices=N_CORES)

    query = nc.dram_tensor("query", [B, LQ, D], f32, kind="ExternalInput")
    context = nc.dram_tensor("context", [B, LC, D], f32, kind="ExternalInput")
    w0t = nc.dram_tensor("w0t", [D, DSL], f16, kind="ExternalInput")
    w1t = nc.dram_tensor("w1t", [D, DSL], f16, kind="ExternalInput")
    w2t = nc.dram_tensor("w2t", [D, DSL], f16, kind="ExternalInput")
    w3t = nc.dram_tensor("w3t", [D, DSL], f16, kind="ExternalInput")
    w4t = nc.dram_tensor("w4t", [D, D], f16, kind="ExternalInput")
    w5t = nc.dram_tensor("w5t", [D, D], f16, kind="ExternalInput")
    b0s = nc.dram_tensor("b0s", [DSL, 1], f32, kind="ExternalInput")
    b1s = nc.dram_tensor("b1s", [DSL, 1], f32, kind="ExternalInput")
    b2r = nc.dram_tensor("b2r", [128, DSL], f32, kind="ExternalInput")
    b3r = nc.dram_tensor("b3r", [128, DSL], f32, kind="ExternalInput")
    b4r = nc.dram_tensor("b4r", [128, D], f32, kind="ExternalInput")
    b5r = nc.dram_tensor("b5r", [128, D], f32, kind="ExternalInput")
    ident = nc.dram_tensor("ident", [128, 128], f16, kind="ExternalInput")
    out0c = nc.dram_tensor("out0c", [LSL, D], f32, kind="ExternalOutput")
    out1c = nc.dram_tensor("out1c", [LSL, D], f32, kind="ExternalOutput")

    with tile.TileContext(nc) as tc:
      for _rep in range(reps):
        with tc.tile_pool(name="dram", bufs=1, space="DRAM") as dram, \
             tc.tile_pool(name="const", bufs=1) as constp, \
             tc.tile_pool(name="psA", bufs=3, space="PSUM") as psA, \
             tc.tile_pool(name="psB", bufs=2, space="PSUM") as psB:

            # fp16 staging of the two activations (cast on SWDGE)
            stage_q = dram.tile([B, LQ, D], f16)
            stage_c = dram.tile([B, LC, D], f16)
            a2aq_in = dram.tile([N_CORES, DSL, LSL], f16)
            a2aq_out = dram.tile([N_CORES, DSL, LSL], f16)
            a2ac_in = dram.tile([N_CORES, DSL, LSL], f16)
            a2ac_out = dram.tile([N_CORES, DSL, LSL], f16)
            # DRAM bounce for the transpose-offloaded exp matrices
            etd = {bh: dram.tile([LC, LQ], f16, name=f"etd{bh[0]}_{bh[1]}")
                   for bh in OFFLOAD}

            for b in range(B):
                nc.gpsimd.dma_start(stage_q[b], query.ap()[b])
                nc.gpsimd.dma_start(stage_c[b], context.ap()[b])

            # constants / weights on the sync queue, before the transposes
            # (they fit in the window while the first cast runs)
            idt = constp.tile([128, 128], f16, name="idt")
            nc.sync.dma_start(idt[:], ident.ap())
            bias_qp = constp.tile([DSL, 1], f32, name="bias_qp")
            nc.sync.dma_start(bias_qp[:], b0s.ap())
            bias_cp = constp.tile([DSL, 1], f32, name="bias_cp")
            nc.sync.dma_start(bias_cp[:], b1s.ap())
            bias_qv = constp.tile([128, DSL], f32, name="bias_qv")
            nc.sync.dma_start(bias_qv[:], b2r.ap())
            bias_cv = constp.tile([128, DSL], f32, name="bias_cv")
            nc.sync.dma_start(bias_cv[:], b3r.ap())
            wq = [constp.tile([128, DSL], f16, name=f"wq{k}") for k in range(NKT)]
            wc = [constp.tile([128, DSL], f16, name=f"wc{k}") for k in range(NKT)]
            wqv = [constp.tile([128, DSL], f16, name=f"wqv{k}")
                   for k in range(NKT)]
            wcv = [constp.tile([128, DSL], f16, name=f"wcv{k}")
                   for k in range(NKT)]
            for k in range(NKT):
                sl = slice(128 * k, 128 * (k + 1))
                nc.sync.dma_start(wq[k][:], w0t.ap()[sl])
                nc.sync.dma_start(wc[k][:], w1t.ap()[sl])
                nc.sync.dma_start(wqv[k][:], w2t.ap()[sl])
                nc.sync.dma_start(wcv[k][:], w3t.ap()[sl])

            # ---- phase 1: transpose inputs + all projections ----
            with tc.tile_pool(name="proj", bufs=1) as projp:
                qTp = [projp.tile([128, LQ], f16, name=f"qTp{b}")
                       for b in range(B)]
                cTp = [projp.tile([128, LC], f16, name=f"cTp{b}")
                       for b in range(B)]
                # merged per-(batch, ltile) value tiles: cols [0:65] head 0
                # (ones at 64), [65:130] head 1 (ones at 129)
                qvv = [[projp.tile([128, 2 * VW], f16, name=f"qvv{b}_{lt}")
                        for lt in range(NLT)] for b in range(B)]
                cvv = [[projp.tile([128, 2 * VW], f16, name=f"cvv{b}_{lt}")
                        for lt in range(NLT)] for b in range(B)]

                with tc.tile_pool(name="inT", bufs=2 * NKT) as inp:
                    for b in range(B):
                        qT = [inp.tile([128, LQ], f16, tag="qT",
                                       name=f"qT{b}_{k}") for k in range(NKT)]
                        cT = [inp.tile([128, LC], f16, tag="cT",
                                       name=f"cT{b}_{k}") for k in range(NKT)]
                        for k in range(NKT):
                            dsl = slice(128 * k, 128 * (k + 1))
                            nc.sync.dma_start(qT[k][:], stage_q[b, :, dsl],
                                              transpose=True)
                            nc.sync.dma_start(cT[k][:], stage_c[b, :, dsl],
                                              transpose=True)
                        for (dst, w_, src, bias) in ((cTp, wc, cT, bias_cp),
                                                     (qTp, wq, qT, bias_qp)):
                            for ch in range(LQ // 512):
                                cs = slice(512 * ch, 512 * (ch + 1))
                                ps = psB.tile([128, 512], f32, tag="pss",
                                              name="ps")
                                for k in range(NKT):
                                    nc.tensor.matmul(ps[:], w_[k][:],
                                                     src[k][:, cs],
                                                     start=(k == 0),
                                                     stop=(k == NKT - 1))
                                nc.vector.tensor_scalar(
                                    out=dst[b][:, cs], in0=ps[:],
                                    scalar1=bias[:, 0:1], scalar2=None, op0=add)
                        for (dst, w_, src, bias) in ((cvv, wcv, cT, bias_cv),
                                                     (qvv, wqv, qT, bias_qv)):
                            for lt in range(NLT):
                                ls = slice(128 * lt, 128 * (lt + 1))
                                ps = psB.tile([128, DSL], f32, tag="pss",
                                              name="ps")
                                for k in range(NKT):
                                    nc.tensor.matmul(ps[:], src[k][:, ls],
                                                     w_[k][:],
                                                     start=(k == 0),
                                                     stop=(k == NKT - 1))
                                t = dst[b][lt]
                                for h in range(HPC):
                                    hs = slice(DK * h, DK * (h + 1))
                                    os = slice(VW * h, VW * h + DK)
                                    nc.vector.tensor_tensor(
                                        out=t[:, os], in0=ps[:, hs],
                                        in1=bias[:, hs], op=add)
                                    nc.vector.memset(
                                        t[:, VW * h + DK:VW * (h + 1)], 1.0)

                # ---- phase 2: attention ----
                with tc.tile_pool(name="att", bufs=1) as attp, \
                     tc.tile_pool(name="emat", bufs=48) as ematp:
                    rq = [[attp.tile([128, 128], f16, name=f"rq{b}_{m}")
                           for m in range(NLT)] for b in range(B)]
                    rc = [[attp.tile([128, 128], f16, name=f"rc{b}_{m}")
                           for m in range(NLT)] for b in range(B)]
                    rqt = [attp.tile([128, LQ], f16, name=f"rqt{b}")
                           for b in range(B)]
                    rct = [attp.tile([128, LC], f16, name=f"rct{b}")
                           for b in range(B)]

                    def scores_exp_packed(b, lhsp, rhsp, qh, dumps):
                        """Both heads' exp(S/sqrt(dk)) for one q-half; the two
                        K=64 score matmuls packed into PE row groups 0/64."""
                        ets = ([], [])
                        for kt in range(NLT):
                            ks = slice(128 * kt, 128 * (kt + 1))
                            sps = [psA.tile([128, 1024], f32, tag="sps",
                                            name="sp") for _ in range(HPC)]
                            for cch in range(2):
                                c0 = 1024 * qh + 512 * cch
                                ds = slice(512 * cch, 512 * (cch + 1))
                                for h in range(HPC):
                                    hp = slice(64 * h, 64 * (h + 1))
                                    nc.tensor.matmul(
                                        sps[h][:, ds], lhsp[hp, ks],
                                        rhsp[hp, c0:c0 + 512],
                                        start=True, stop=True)
                            for h in range(HPC):
                                e = ematp.tile([128, 1024], f16, tag="et",
                                               name="e")
                                nc.scalar.activation(e[:], sps[h][:], Exp,
                                                     scale=SCALE)
                                if dumps[h] is not None:
                                    nc.gpsimd.dma_start(
                                        dumps[h][ks,
                                                 1024 * qh:1024 * (qh + 1)],
                                        e[:])
                                ets[h].append(e)
                        return ets

                    def scores_exp_half(b, h, lhsp, rhsp, qh):
                        """One head's exp tiles for one half (V phase)."""
                        hp = slice(64 * h, 64 * (h + 1))
                        et = []
                        for kt in range(NLT):
                            ks = slice(128 * kt, 128 * (kt + 1))
                            sp = psA.tile([128, 1024], f32, tag="sps",
                                          name="sp")
                            for cch in range(2):
                                c0 = 1024 * qh + 512 * cch
                                ds = slice(512 * cch, 512 * (cch + 1))
                                nc.tensor.matmul(sp[:, ds], lhsp[hp, ks],
                                                 rhsp[hp, c0:c0 + 512],
                                                 start=True, stop=True)
                            e = ematp.tile([128, 1024], f16, tag="et", name="e")
                            nc.scalar.activation(e[:], sp[:], Exp, scale=SCALE)
                            et.append(e)
                        return et

                    def xbar_half(bh, ch):
                        """V-phase E tiles for one c-half via xbar transpose."""
                        et = []
                        for kt in range(NLT):
                            ks = slice(128 * kt, 128 * (kt + 1))
                            e = ematp.tile([128, 1024], f16, tag="et", name="e")
                            nc.sync.dma_start(
                                e[:], etd[bh][1024 * ch:1024 * (ch + 1), ks],
                                transpose=True)
                            et.append(e)
                        return et

                    def apply_norm_half(et, vals, h, rdst, mh):
                        hp = slice(64 * h, 64 * (h + 1))
                        vs = slice(VW * h, VW * (h + 1))
                        for mi in range(8):
                            m = 8 * mh + mi
                            up = psB.tile([128, VW], f32, tag="pss", name="up")
                            for kt in range(NLT):
                                nc.tensor.matmul(
                                    up[:], et[kt][:, 128 * mi:128 * (mi + 1)],
                                    vals[kt][:, vs],
                                    start=(kt == 0), stop=(kt == NLT - 1))
                            rec = attp.tile([128, 1], f32, tag="rec", bufs=4,
                                            name="rec")
                            nc.vector.reciprocal(rec[:], up[:, DK:DK + 1])
                            nc.vector.tensor_scalar(
                                out=rdst[m][:, hp], in0=up[:, 0:DK],
                                scalar1=rec[:, 0:1], scalar2=None, op0=mult)

                    def shard_out(r, rt, b, a2a_in):
                        for m in range(NLT):
                            ms = slice(128 * m, 128 * (m + 1))
                            tp = psB.tile([128, 128], f16, tag="pss", name="tp")
                            nc.tensor.transpose(tp[:], r[b][m][:], idt[:])
                            nc.vector.tensor_copy(rt[b][:, ms], tp[:])
                        for j in range(4):
                            js = slice(512 * j, 512 * (j + 1))
                            nc.gpsimd.dma_start(a2a_in[4 * b + j], rt[b][:, js])

                    # U phase (row softmax -> q_res), both heads packed in the
                    # PE array; exp(S^T) halves of offloaded pairs go to DRAM
                    for b in range(B):
                        dumps = [etd.get((b, h)) for h in range(HPC)]
                        for qh in range(2):
                            ets = scores_exp_packed(b, cTp[b], qTp[b], qh,
                                                    dumps)
                            for h in range(HPC):
                                apply_norm_half(ets[h], cvv[b], h, rq[b], qh)
                        shard_out(rq, rqt, b, a2aq_in)

                    nc.gpsimd.collective_compute(
                        "AllToAll", mybir.AluOpType.bypass,
                        replica_groups=[list(range(N_CORES))],
                        ins=[a2aq_in.opt()], outs=[a2aq_out.opt()])

                    # V phase (col softmax -> c_res), non-offloaded head first
                    for b in range(B):
                        for h in sorted(range(HPC),
                                        key=lambda h_: (b, h_) in OFFLOAD):
                            for ch in range(2):
                                if (b, h) in OFFLOAD:
                                    et = xbar_half((b, h), ch)
                                else:
                                    et = scores_exp_half(b, h, qTp[b], cTp[b],
                                                         ch)
                                apply_norm_half(et, qvv[b], h, rc[b], ch)
                        shard_out(rc, rct, b, a2ac_in)

                    # out_q projection, hidden under the V phase: weights are
                    # streamed from DRAM through a small rotating pool
                    with tc.tile_pool(name="o0p", bufs=1) as o0p:
                        rqf = [o0p.tile([128, LSL], f16, name=f"rqf{k}")
                               for k in range(NKT)]
                        for k in range(NKT):
                            nc.gpsimd.dma_start(rqf[k][:], a2aq_out[k])
                        bias4 = o0p.tile([128, D], f32, name="bias4")
                        nc.gpsimd.dma_start(bias4[:], b4r.ap())
                        for mt in range(LSL // 128):
                            ms = slice(128 * mt, 128 * (mt + 1))
                            for ch in range(D // 512):
                                cs = slice(512 * ch, 512 * (ch + 1))
                                ps = psB.tile([128, 512], f32, tag="pss",
                                              name="ps")
                                for k in range(NKT):
                                    wk = o0p.tile([128, 512], f16, tag="w4s",
                                                  bufs=4, name="wk")
                                    nc.sync.dma_start(
                                        wk[:],
                                        w4t.ap()[128 * k:128 * (k + 1), cs])
                                    nc.tensor.matmul(ps[:], rqf[k][:, ms],
                                                     wk[:], start=(k == 0),
                                                     stop=(k == NKT - 1))
                                ev = o0p.tile([128, 512], f32, tag="oev",
                                              bufs=3, name="ev")
                                nc.vector.tensor_tensor(out=ev[:], in0=ps[:],
                                                        in1=bias4[:, cs],
                                                        op=add)
                                nc.gpsimd.dma_start(out0c.ap()[ms, cs], ev[:])

                    nc.gpsimd.collective_compute(
                        "AllToAll", mybir.AluOpType.bypass,
                        replica_groups=[list(range(N_CORES))],
                        ins=[a2ac_in.opt()], outs=[a2ac_out.opt()])

            # ---- phase 3: out_c projection ----
            with tc.tile_pool(name="outp", bufs=1) as outp:
                w5 = [outp.tile([128, D], f16, name=f"w5_{k}")
                      for k in range(NKT)]
                bias5 = outp.tile([128, D], f32, name="bias5")
                nc.sync.dma_start(bias5[:], b5r.ap())
                for k in range(NKT):
                    sl = slice(128 * k, 128 * (k + 1))
                    nc.sync.dma_start(w5[k][:], w5t.ap()[sl])
                rcf = [outp.tile([128, LSL], f16, name=f"rcf{k}")
                       for k in range(NKT)]
                for k in range(NKT):
                    nc.sync.dma_start(rcf[k][:], a2ac_out[k])

                for mt in range(LSL // 128):
                    ms = slice(128 * mt, 128 * (mt + 1))
                    for ch in range(D // 512):
                        cs = slice(512 * ch, 512 * (ch + 1))
                        ps = psB.tile([128, 512], f32, tag="pss", name="ps")
                        for k in range(NKT):
                            nc.tensor.matmul(ps[:], rcf[k][:, ms],
                                             w5[k][:, cs],
                                             start=(k == 0),
                                             stop=(k == NKT - 1))
                        ev = outp.tile([128, 512], f32, tag="oev", bufs=3,
                                       name="ev")
                        nc.vector.tensor_tensor(out=ev[:], in0=ps[:],
                                                in1=bias5[:, cs], op=add)
                        nc.sync.dma_start(out1c.ap()[ms, cs], ev[:])

    nc.compile()
    return nc


def _prep_inputs(inputs):
    f16 = np.float16
    f32 = np.float32
    q = np.ascontiguousarray(np.asarray(inputs["query"], dtype=f32))
    c = np.ascontiguousarray(np.asarray(inputs["context"], dtype=f32))
    W = [np.asarray(inputs[f"W{i}"], dtype=f32) for i in range(6)]
    bias = [np.asarray(inputs[f"b{i}"], dtype=f32) for i in range(6)]
    ident = np.eye(128, dtype=f16)
    in_maps = []
    for k in range(N_CORES):
        dsl = slice(DSL * k, DSL * (k + 1))
        m = {
            "query": q,
            "context": c,
            "w0t": np.ascontiguousarray(W[0][dsl].T.astype(f16)),
            "w1t": np.ascontiguousarray(W[1][dsl].T.astype(f16)),
            "w2t": np.ascontiguousarray(W[2][dsl].T.astype(f16)),
            "w3t": np.ascontiguousarray(W[3][dsl].T.astype(f16)),
            "w4t": np.ascontiguousarray(W[4].T.astype(f16)),
            "w5t": np.ascontiguousarray(W[5].T.astype(f16)),
            "b0s": np.ascontiguousarray(bias[0][dsl].reshape(DSL, 1)),
            "b1s": np.ascontiguousarray(bias[1][dsl].reshape(DSL, 1)),
            "b2r": np.ascontiguousarray(np.tile(bias[2][dsl], (128, 1))),
            "b3r": np.ascontiguousarray(np.tile(bias[3][dsl], (128, 1))),
            "b4r": np.ascontiguousarray(np.tile(bias[4], (128, 1))),
            "b5r": np.ascontiguousarray(np.tile(bias[5], (128, 1))),
            "ident": ident,
        }
        in_maps.append(m)
    return in_maps


def _get_program(reps=1):
    key = f"nc{reps}"
    if key not in _CACHE:
        _CACHE[key] = _build_program(reps)
    return _CACHE[key]


def _get_runner():
    """Build (once) a reusable sharded PJRT callable for the program so
    repeated kernel() calls don't re-trace/re-compile the XLA wrapper."""
    if "runner" in _CACHE:
        return _CACHE["runner"]
    import jax
    from jax.sharding import Mesh, PartitionSpec, NamedSharding
    from jax.experimental.shard_map import shard_map
    import concourse.mybir as mybir
    from concourse.bass2jax import (_bass_exec_p, partition_id_tensor,
                                    install_neuronx_cc_hook)

    nc = _get_program()
    install_neuronx_cc_hook()
    partition_name = (nc.partition_id_tensor.name
                      if nc.partition_id_tensor else None)
    in_names, out_names, out_avals, zero_outs = [], [], [], []
    for alloc in nc.m.functions[0].allocations:
        if not isinstance(alloc, mybir.MemoryLocationSet):
            continue
        name = alloc.memorylocations[0].name
        if alloc.kind == "ExternalInput":
            if name != partition_name:
                in_names.append(name)
        elif alloc.kind == "ExternalOutput":
            out_names.append(name)
            shape = tuple(alloc.tensor_shape)
            dtype = mybir.dt.np(alloc.dtype)
            out_avals.append(jax.core.ShapedArray(shape, dtype))
            zero_outs.append(np.zeros(shape, dtype))
    n_params = len(in_names)
    all_in = list(in_names) + list(out_names)
    if partition_name is not None:
        all_in.append(partition_name)

    def _body(*args):
        operands = list(args)
        if partition_name is not None:
            operands.append(partition_id_tensor())
        return tuple(_bass_exec_p.bind(
            *operands, out_avals=tuple(out_avals), in_names=tuple(all_in),
            out_names=tuple(out_names), lowering_input_output_aliases=(),
            sim_require_finite=True, sim_require_nnan=True, nc=nc))

    devices = jax.devices()[:N_CORES]
    mesh = Mesh(np.asarray(devices), ("core",))
    nspec = (PartitionSpec("core"),)
    fn = jax.jit(shard_map(_body, mesh=mesh,
                           in_specs=nspec * (n_params + len(out_names)),
                           out_specs=nspec * len(out_names), check_rep=False),
                 keep_unused=True)
    sharding = NamedSharding(mesh, PartitionSpec("core"))
    zeros_staged = [
        jax.device_put(np.concatenate([z] * N_CORES, axis=0), sharding)
        for z in zero_outs]

    def run(in_maps):
        concat = [np.concatenate([np.asarray(in_maps[c][n])
                                  for c in range(N_CORES)], axis=0)
                  for n in in_names]
        staged = [jax.device_put(a, sharding) for a in concat]
        outs = fn(*staged, *zeros_staged)
        res = []
        for c in range(N_CORES):
            res.append({name: np.asarray(outs[i]).reshape(
                N_CORES, *out_avals[i].shape)[c]
                for i, name in enumerate(out_names)})
        return res

    _CACHE["runner"] = run
    return run


def kernel(**inputs):
    run = _get_runner()
    res = run(_prep_inputs(inputs))
    out0 = np.concatenate([res[k]["out0c"] for k in range(N_CORES)], axis=0)
    out1 = np.concatenate([res[k]["out1c"] for k in range(N_CORES)], axis=0)
    return (out0.reshape(B, LQ, D).astype(np.float32),
            out1.reshape(B, LC, D).astype(np.float32))
